# revision 42
# baseline (speedup 1.0000x reference)
"""Causal grouped Conv1d on 8 Trainium2 NeuronCores.

Problem: x [B=4, L=4096, D=2048] f32, w [K=4, D/G=256, D=2048] f32, G=8 groups.
out[b, l, o] = sum_{k, i} x[b, l-3+k, g(o)*256 + i] * w[k, i, o]   (causal pad 3)

Sharding: hybrid tensor/data parallel — core c = (th, gp) with th = c // 4,
gp = c % 4 handles batches {2*th, 2*th+1} x channel slice [gp*512, gp*512+512)
(= groups 2*gp, 2*gp+1). Per-core w is 2 MB (no weight-vs-x HBM contention at
startup) and x rows stay 2 KB contiguous (efficient DMA descriptors).

The host packs each core's x slice as 2 batches of [4 zero halo rows + 4096
token rows] so causal padding at batch starts falls out naturally.

Per-core kernel (Bass/Tile):
  - PE-transpose x into xT tiles [128 cin, 515 tok] per 512-token block
    (float32r transposes, 1.5 cyc/row)
  - conv matmuls (float32r, moving N=512): psum[128 och, 512 tok] +=
    w[k, ci, och].T @ xT[:, k:k+512] accumulated over k and 2 cin chunks
  - output written och-major [512, 8192] to HBM; host transposes back.
"""

import numpy as np

import concourse.bass as bass
import concourse.mybir as mybir
import concourse.tile as tile
from concourse import bacc
from concourse.bass_utils import run_bass_kernel_spmd

B, L, D, K, G = 4, 4096, 2048, 4, 8
CG = D // G               # 256 channels per group (in and out)
NCORES = 8
BPC = 2                   # batches per core
CPC = 512                 # channels per core (2 groups)
TOKC = BPC * L            # 8192 tokens per core
TT = 128                  # row tile for DMA/transpose
NCHUNK = CPC // 128       # 4 cin chunks of 128 per core
PAD = K - 1               # 3 (causal left pad)
HPAD = 4                  # halo rows fetched (fp32r transpose needs even cols)
ROWS_PER_B = HPAD + L     # shard rows per batch

F32 = mybir.dt.float32
F32R = mybir.dt.float32r
MM_F32R = True            # conv matmuls + transposes in float32r

TB = 512                  # token block for the matmul moving dim
NB_PER_B = L // TB        # 8 blocks per batch
NB = BPC * NB_PER_B       # 16 blocks per core


def _emit(tc, nc, xs, wt, consts, y):
    """xs [BPC*(HPAD+L), CPC]; wt [K, CG, CPC]; y [CPC, TOKC] (och-major)."""
    import contextlib
    ctx = contextlib.ExitStack()
    mmdt = F32R if MM_F32R else F32
    with ctx:
        constp = ctx.enter_context(tc.tile_pool(name="constp", bufs=1))
        wp = ctx.enter_context(tc.tile_pool(name="wp", bufs=1))
        xinp = ctx.enter_context(tc.tile_pool(name="xinp", bufs=5))
        xtp = ctx.enter_context(tc.tile_pool(name="xtp", bufs=12))
        outp = ctx.enter_context(tc.tile_pool(name="outp", bufs=8))
        pm = ctx.enter_context(tc.tile_pool(name="pm", bufs=3, space="PSUM"))
        po = ctx.enter_context(tc.tile_pool(name="po", bufs=5, space="PSUM"))

        # consts input: cols 0-127 = identity, cols 128-131 = zeros
        cst = constp.tile([128, 132], mmdt)
        nc.sync.dma_start(cst[:], consts[:])
        ident = cst[:, 0:128]
        zero = cst[:, 128:132]

        def row0(t):
            """Shard row of token t*TB (start of block t)."""
            bi, tb = divmod(t, NB_PER_B)
            return bi * ROWS_PER_B + HPAD + tb * TB

        def issue_x(t):
            r0 = row0(t)
            # One DMA for all 512 rows: [512, CPC] -> [128, 4, CPC]
            xm = xinp.tile([TT, (TB // TT) * CPC], mmdt, name="xm")
            nc.sync.dma_start(
                xm.rearrange("p (i c) -> p i c", c=CPC),
                xs[r0: r0 + TB, :].rearrange("(i p) c -> p i c", p=TT),
            )
            return xm

        # Startup DMA order on the SP HWDGE FIFO (FIFO = arrival order):
        # x block 0, w och-quarters 0-1, x block 1, w quarters 2-3 — so conv
        # chunk cc's weights land just before its first matmuls.
        wsb = {}
        for k in range(K):
            for j in range(2):
                wsb[(k, j)] = wp.tile([128, CPC], mmdt, name=f"w_{k}_{j}")

        def dma_w_quarter(q):
            qs = slice(q * 128, (q + 1) * 128)
            for k in range(K):
                for j in range(2):
                    nc.sync.dma_start(
                        wsb[(k, j)][:, qs], wt[k, j * 128:(j + 1) * 128, qs]
                    )

        pending = {0: issue_x(0)}
        dma_w_quarter(0)
        dma_w_quarter(1)
        pending[1] = issue_x(1)
        dma_w_quarter(2)
        dma_w_quarter(3)

        prev_xts = None  # chunk -> xT tile of the previous block
        for t in range(NB):
            t0 = t * TB
            xm = pending.pop(t)
            if t + 2 < NB:
                pending[t + 2] = issue_x(t + 2)
            batch_start = t % NB_PER_B == 0

            xts = []
            for c in range(NCHUNK):
                # pmt cols = tokens [t0, t0+TB)
                pmt = pm.tile([128, TB], mmdt, name="pmt")
                for i in range(TB // TT):
                    nc.tensor.transpose(
                        pmt[:, i * TT:(i + 1) * TT],
                        xm[:, i * CPC + c * 128: i * CPC + (c + 1) * 128],
                        ident[:],
                    )
                # xt cols = tokens [t0-3, t0+TB); halo comes from the tail of
                # the previous block's xt (zeros at batch start).
                xt_t = xtp.tile([128, TB + PAD], mmdt, name="xt_t")
                nc.vector.tensor_copy(xt_t[:, PAD:], pmt[:])
                if batch_start:
                    nc.vector.tensor_copy(xt_t[:, 0:PAD], zero[:, 0:PAD])
                else:
                    nc.vector.tensor_copy(
                        xt_t[:, 0:PAD], prev_xts[c][:, TB:TB + PAD]
                    )
                xts.append(xt_t)

            for cc in range(NCHUNK):
                gg = cc // 2  # local group of this och chunk
                pot = po.tile([128, TB], F32, name="pot")
                first = True
                for j in range(2):
                    xt_t = xts[2 * gg + j]
                    for k in range(K):
                        nc.tensor.matmul(
                            pot[:],
                            wsb[(k, j)][:, cc * 128:(cc + 1) * 128],
                            xt_t[:, k:k + TB],
                            start=first,
                            stop=(j == 1 and k == K - 1),
                        )
                        first = False
                ot = outp.tile([128, TB], F32, name="ot")
                if cc % 2 == 0:
                    nc.vector.tensor_copy(ot[:], pot[:])
                else:
                    nc.scalar.copy(ot[:], pot[:])
                nc.sync.dma_start(
                    y[cc * 128:(cc + 1) * 128, t0:t0 + TB], ot[:]
                )
            prev_xts = xts


_NC_CACHE = None


def build_nc():
    global _NC_CACHE
    if _NC_CACHE is not None:
        return _NC_CACHE
    mmdt = F32R if MM_F32R else F32
    nc = bacc.Bacc(
        "TRN2", target_bir_lowering=False, debug=False, num_devices=NCORES
    )
    xs = nc.dram_tensor(
        "xs", [BPC * ROWS_PER_B, CPC], mmdt, kind="ExternalInput"
    ).ap()
    wt = nc.dram_tensor("wt", [K, CG, CPC], mmdt, kind="ExternalInput").ap()
    consts = nc.dram_tensor("consts", [128, 132], mmdt, kind="ExternalInput").ap()
    y = nc.dram_tensor("y", [CPC, TOKC], F32, kind="ExternalOutput").ap()
    with tile.TileContext(nc) as tc:
        _emit(tc, nc, xs, wt, consts, y)
    nc.compile()
    _NC_CACHE = nc
    return nc


def make_in_maps(x, w):
    """Slice x/w per core; pack x as BPC x [HPAD zero rows + L rows]."""
    x = np.ascontiguousarray(x, dtype=np.float32)
    w = np.ascontiguousarray(w, dtype=np.float32)
    consts = np.concatenate(
        [np.eye(128, dtype=np.float32), np.zeros((128, 4), np.float32)], axis=1
    )
    in_maps = []
    for core in range(NCORES):
        th, gp = divmod(core, 4)
        cs = slice(gp * CPC, (gp + 1) * CPC)
        shard = np.zeros((BPC * ROWS_PER_B, CPC), dtype=np.float32)
        for bb in range(BPC):
            b = BPC * th + bb
            shard[bb * ROWS_PER_B + HPAD:(bb + 1) * ROWS_PER_B] = x[b, :, cs]
        in_maps.append(
            {"xs": shard, "wt": np.ascontiguousarray(w[:, :, cs]), "consts": consts}
        )
    return in_maps


def run(x, w, trace=False, **kw):
    nc = build_nc()
    res = run_bass_kernel_spmd(
        nc, make_in_maps(x, w), core_ids=list(range(NCORES)), trace=trace, **kw
    )
    out = np.empty((B, L, D), dtype=np.float32)
    for core in range(NCORES):
        th, gp = divmod(core, 4)
        yc = res.results[core]["y"]  # [CPC, BPC*L]
        out[BPC * th: BPC * (th + 1), :, gp * CPC:(gp + 1) * CPC] = (
            yc.reshape(CPC, BPC, L).transpose(1, 2, 0)
        )
    return out, res


def kernel(x, w):
    out, _ = run(x, w, trace=False)
    return out
